# revision 61
# baseline (speedup 1.0000x reference)
"""3-layer GCN + mean-pool + FFN + softmax on 8 Trainium2 NeuronCores.

Strategy (node sharding, per the edge-partitioning hint):
  - Nodes are sharded across 8 cores by id range (12500 each); within a core,
    nodes occupy 12544 slots laid out as 128 partitions x 98 chunks.
  - Per layer: y = dinv * (h @ W) chunk-wise (PE transpose + matmul; the dinv
    pre-scale for the NEXT layer is folded into the relu epilogue so P1 is a
    plain matmul), AllGather of y (bf16) across cores, then edge aggregation:
    edges are grouped per (dst core, src window) and each destination's edge
    count is decomposed into quads/pairs/singles.  Quad cells gather 4
    same-dst messages into 4 adjacent columns of one partition, two strided
    DVE adds reduce them 4->1, and only the survivors go through the CCE
    scatter-add -- directly into SBUF accumulators (parity-split mode), not
    DRAM.  This cuts scatter descriptors ~2.3x and removes all DRAM RMW.
  - Graph mean-pool via a matmul with a host-built membership/count matrix,
    AllReduce of the [16,128] partial, then FFN + softmax on-chip.
"""
import numpy as np

import concourse.bass as bass
import concourse.mybir as mybir
import concourse.tile as tile
from concourse import bacc
from concourse.bass_utils import run_bass_kernel_spmd
from concourse.masks import make_identity

NCORES = 8
N_FULL, E_FULL, G_FULL, D_FULL, C_FULL = 100000, 1600000, 16, 128, 16

f32 = mybir.dt.float32
i32 = mybir.dt.int32
i16 = mybir.dt.int16

WIN = 25088          # = 2 owner slabs, so window gathers chase slab writes
CSZ = {"q": 4, "p": 2, "s": 1}
# max gather idx per chunk (gat tile = IDX_PER_CHUNK rows x 256B)
IDX_PER_CHUNK = 3072


def wrap16(a):
    w16 = a.reshape(-1, 16).T.copy()
    return np.tile(w16, (8, 1))


def host_prep(x, edge_index, batch, n, g, d, ncores):
    """Build per-core slot layouts, quad/pair/single gather cells, pooling.

    Slots are split into two halves (A: chunks < rA, B: the rest), with pad
    slots in BOTH halves, and y lives in two shared tensors so each half's
    AllGather can overlap the other half's edge work."""
    np_ = (n + ncores - 1) // ncores          # nodes per core
    rA = min(48, max(2, (np_ // 128 + 1) // 2 // 2 * 2))
    slotsA = rA * 128
    padA = max(1, min(44, slotsA // 16))
    realA = min(np_, slotsA - padA)
    realB = np_ - realA
    assert realB > 0, "half split needs nodes in both halves"
    rB = (realB + 16 + 127) // 128
    r_ = rA + rB
    slots = r_ * 128
    rh = (r_ + 1) // 2

    dst_full = np.concatenate([np.asarray(edge_index[1]),
                               np.arange(n, dtype=np.int64)])
    deg = np.bincount(dst_full, minlength=n).astype(np.float32)
    dinv = np.where(deg > 0, 1.0 / np.sqrt(np.maximum(deg, 1e-30)),
                    0.0).astype(np.float32)

    node_core = np.minimum(np.arange(n) // np_, ncores - 1)
    local = np.arange(n) - node_core * np_
    rank = np.where(local < realA, local, local - realA + slotsA)
    # y row of node i: half A rows are q*rA + r, half B rows q*rB + (r - rA)
    rr = rank // 128
    qq = node_core * 128 + rank % 128
    in_a = rr < rA
    yrow = np.where(in_a, qq * rA + rr, qq * rB + (rr - rA))

    # per-core slot-ordered x, xT, dinv, batch (pads zero / -1)
    x_slot = np.zeros((ncores, slots, d), np.float32)
    dinv_slot = np.zeros((ncores, slots), np.float32)
    batch_slot = np.full((ncores, slots), -1, np.int64)
    flat = node_core * slots + rank
    x_slot.reshape(ncores * slots, d)[flat] = np.asarray(x, np.float32)
    dinv_slot.reshape(-1)[flat] = dinv
    batch_slot.reshape(-1)[flat] = np.asarray(batch)

    def to_pr(a):  # [ncores, slots, ...] -> [ncores, 128, r_ * ...]
        rest = a.shape[2:]
        m = int(np.prod(rest)) if rest else 1
        return (a.reshape(ncores, r_, 128, m).transpose(0, 2, 1, 3)
                 .reshape(ncores, 128, r_ * m).copy())

    dinv_pr = to_pr(dinv_slot[..., None])
    dinv2_pr = dinv_pr * dinv_pr
    # full transposed PRE-SCALED x (dinv*x), owner-major, bf16: every core
    # computes y1 for ALL nodes locally (kills the layer-1 AllGather)
    import concourse.mybir as _mb
    bfnp = _mb.dt.np(_mb.dt.bfloat16)
    xs_slot = x_slot * dinv_slot[..., None]
    xTf = (xs_slot.reshape(ncores, r_, 128, d).transpose(3, 0, 1, 2)
           .reshape(d, ncores * r_ * 128).astype(bfnp))
    # per-core copy of its own slab (the "9th pass" recomputes y_sb locally,
    # since SPMD code cannot address "my" slab inside the replicated stream)
    xT_pr = (xs_slot.reshape(ncores, r_, 128, d).transpose(0, 3, 1, 2)
             .reshape(ncores, d, r_ * 128).astype(bfnp))

    cnt = np.bincount(np.asarray(batch), minlength=g).astype(np.float32)
    cntc = np.clip(cnt, 1.0, None)
    onehot = (batch_slot[..., None] == np.arange(g)[None, None, :]
              ).astype(np.float32)
    mp = onehot / cntc[None, None, :]
    mp_pr = to_pr(mp)

    # --- edge cells: per (dst half, src tensor-window), quad/pair/single ---
    # (self-loops are handled by the on-chip init copy of the core's own y)
    e_src = np.asarray(edge_index[0]).astype(np.int64)
    e_dst = np.asarray(edge_index[1]).astype(np.int64)
    dst_core = node_core[e_dst]
    dst_rank = rank[e_dst]
    src_row = yrow[e_src]
    src_in_a = in_a[e_src]

    # tensor-windows: split each tensor's row space into <=32767-row windows
    wins = []                                 # (tensor, lo, hi)
    for tname, rows in (("A", ncores * 128 * rA), ("B", ncores * 128 * rB)):
        nw = (rows + 32767) // 32768
        wsz = (rows + nw - 1) // nw
        wsz = (wsz + 127) // 128 * 128
        for k in range(nw):
            lo, hi = k * wsz, min((k + 1) * wsz, rows)
            if lo < hi:
                wins.append((tname, lo, hi))
    # edge -> window index
    src_wi = np.full(len(e_src), -1, np.int64)
    for wi, (tname, lo, hi) in enumerate(wins):
        m = (src_in_a if tname == "A" else ~src_in_a) \
            & (src_row >= lo) & (src_row < hi)
        src_wi[m] = wi
    assert (src_wi >= 0).all()
    dst_half = (dst_rank >= slotsA).astype(np.int64)
    # scatter pad ranks, half-relative
    pad_rel = {0: slotsA - 1, 1: slots - 1 - slotsA}

    # chunk emit order: A-src blocks first (they only need AllGather-A),
    # and within each src tensor the H1-dst cells first (so the H1 relu +
    # next AllGather-A can launch while H2 edge work continues).
    keys = []
    for tgrp in ("A", "B"):
        for half in (0, 1):
            for wi, (tname, lo, hi) in enumerate(wins):
                if tname == tgrp:
                    keys.append((half, wi))

    # per (core, key, region): cells (round j, half-relative rank, srcs)
    per_cwr = {}
    for c in range(ncores):
        m_c = dst_core == c
        sr_c = src_row[m_c]
        rk_c = dst_rank[m_c]
        wi_c = src_wi[m_c]
        hf_c = dst_half[m_c]
        for half, wi in keys:
            lo = wins[wi][1]
            mm = (wi_c == wi) & (hf_c == half)
            sr = sr_c[mm] - lo
            rk = rk_c[mm] - half * slotsA
            o = np.argsort(rk, kind="stable")
            sr, rk = sr[o], rk[o]
            uniq, starts, cnts = np.unique(rk, return_index=True,
                                           return_counts=True)
            for region in ("q", "p", "s"):
                per_cwr[(c, half, wi, region)] = []
            for u, s0, cn in zip(uniq, starts, cnts):
                srcs = sr[s0:s0 + cn]
                pos = 0
                nq = cn // 4
                for j in range(nq):
                    per_cwr[(c, half, wi, "q")].append((j, u,
                                                        srcs[pos:pos + 4]))
                    pos += 4
                if (cn - pos) >= 2:
                    per_cwr[(c, half, wi, "p")].append((0, u,
                                                        srcs[pos:pos + 2]))
                    pos += 2
                if cn - pos:
                    per_cwr[(c, half, wi, "s")].append((0, u,
                                                        srcs[pos:pos + 1]))

    # Common chunk structure across cores. For each (key, region): rounds
    # with common (max-over-core) sizes padded to 128 cells, split into
    # pieces and grouped into chunks of <= IDX_PER_CHUNK gather indices.
    chunks = []
    gpos = spos = 0
    for half, wi in keys:
        for region in ("q", "p", "s"):
            csz = CSZ[region]
            cells_pc = IDX_PER_CHUNK // csz
            nrounds = 0
            by_core = []
            for c in range(ncores):
                cells = per_cwr[(c, half, wi, region)]
                nr = 1 + max((j for j, _, _ in cells), default=-1)
                nrounds = max(nrounds, nr)
                by_core.append(cells)
            if nrounds == 0:
                continue
            rsz = np.zeros(nrounds, np.int64)
            for c in range(ncores):
                cnt_r = np.bincount([j for j, _, _ in by_core[c]],
                                    minlength=nrounds)
                rsz = np.maximum(rsz, cnt_r)
            rsz = (rsz + 127) // 128 * 128
            pieces = []
            for j, sz in enumerate(rsz):
                sz = int(sz)
                while sz > cells_pc:
                    pieces.append((j, cells_pc))
                    sz -= cells_pc
                if sz:
                    pieces.append((j, sz))
            cur, cur_cells = [], 0
            groups = []
            for j, sz in pieces:
                if cur and cur_cells + sz > cells_pc:
                    groups.append(cur)
                    cur, cur_cells = [], 0
                cur.append((j, sz))
                cur_cells += sz
            if cur:
                groups.append(cur)
            for grp in groups:
                ncell = sum(sz for _, sz in grp)
                tname, lo, hi = wins[wi]
                chunks.append(dict(half=half, wi=wi, tensor=tname,
                                   lo=lo, hi=hi, region=region,
                                   pieces=[sz for _, sz in grp],
                                   rounds=[j for j, _ in grp],
                                   gpos=gpos, spos=spos, ncell=ncell))
                gpos += ncell * csz
                spos += ncell
    total_gidx, total_sidx = gpos, spos

    # fill per-core index arrays (scatter pads go to the half's pad slot;
    # gather pads read row 0 of the window -- the value is irrelevant since
    # pad slots are zeroed by dinv==0 in the relu epilogue)
    gidx = np.zeros((ncores, total_gidx), np.int16)
    sidx = np.zeros((ncores, total_sidx), np.int16)
    for c in range(ncores):
        cursor = {}
        for ch in chunks:
            half, wi, region = ch["half"], ch["wi"], ch["region"]
            csz = CSZ[region]
            cells = per_cwr[(c, half, wi, region)]
            base_g, base_s = ch["gpos"], ch["spos"]
            cell_off = 0
            for j, sz in zip(ch["rounds"], ch["pieces"]):
                key = (half, wi, region, j)
                st = cursor.get(key, 0)
                sub = [cl for cl in cells if cl[0] == j][st:st + sz]
                cursor[key] = st + len(sub)
                for k, (_, rk, srcs) in enumerate(sub):
                    cc = cell_off + k
                    p, q = cc % 128, cc // 128
                    sidx[c, base_s + cc] = rk
                    for t in range(csz):
                        gidx[c, base_g + (q * csz + t) * 128 + p] = srcs[t]
                for k in range(len(sub), sz):
                    cc = cell_off + k
                    sidx[c, base_s + cc] = pad_rel[half]
                cell_off += sz
        # all cells consumed?
        for half, wi in keys:
            for region in ("q", "p", "s"):
                cells = per_cwr[(c, half, wi, region)]
                nr = 1 + max((j for j, _, _ in cells), default=-1)
                used = sum(cursor.get((half, wi, region, j), 0)
                           for j in range(nr))
                assert used == len(cells), (c, half, wi, region, used,
                                            len(cells))

    gidx_pr = np.stack([wrap16(gidx[c]) for c in range(ncores)])
    sidx_pr = np.stack([wrap16(sidx[c]) for c in range(ncores)])

    return dict(xTf_pr=xTf, xT_pr=xT_pr, dinv_pr=dinv_pr, dinv2_pr=dinv2_pr,
                mp_pr=mp_pr,
                gidx_pr=gidx_pr, sidx_pr=sidx_pr, chunks=chunks,
                total_gidx=total_gidx, total_sidx=total_sidx,
                r_=r_, rh=rh, rA=rA, rB=rB)


def build_gcn(nc, *, r_, rh, chunks, total_gidx, total_sidx, rA, rB, d, g,
              c_, ncores, use_bias, use_fbias, n_layers=3, ydt=None):
    if ydt is None:
        ydt = mybir.dt.bfloat16
    rg = [list(range(ncores))]

    bf16 = mybir.dt.bfloat16
    xTf_in = nc.dram_tensor("xTf_pr", [d, ncores * r_ * 128], bf16,
                            kind="ExternalInput")
    xT_in = nc.dram_tensor("xT_pr", [d, r_ * 128], bf16,
                           kind="ExternalInput")
    w0b_in = nc.dram_tensor("w0b", [d, d], bf16, kind="ExternalInput")
    dinv_in = nc.dram_tensor("dinv_pr", [128, r_], f32, kind="ExternalInput")
    dinv2_in = nc.dram_tensor("dinv2_pr", [128, r_], f32,
                              kind="ExternalInput")
    gidx_in = nc.dram_tensor("gidx_pr", [128, total_gidx // 16], i16,
                             kind="ExternalInput")
    sidx_in = nc.dram_tensor("sidx_pr", [128, total_sidx // 16], i16,
                             kind="ExternalInput")
    mp_in = nc.dram_tensor("mp_pr", [128, r_ * g], f32, kind="ExternalInput")
    w_ins = [nc.dram_tensor(f"w{i}", [d, d], f32, kind="ExternalInput")
             for i in range(3)]
    wf_in = nc.dram_tensor("wf", [d, c_], f32, kind="ExternalInput")
    b_ins = [nc.dram_tensor(f"b{i}", [128, d], f32, kind="ExternalInput")
             for i in range(3)] if use_bias else None
    bf_in = (nc.dram_tensor("bfr", [g, c_], f32, kind="ExternalInput")
             if use_fbias else None)
    out_ext = nc.dram_tensor("out", [g, c_], f32, kind="ExternalOutput")

    y_cA = nc.dram_tensor("y_cA", [128, rA * d], ydt)
    y_cB = nc.dram_tensor("y_cB", [128, rB * d], ydt)
    y_allA = nc.dram_tensor("y_allA", [ncores * 128, rA * d], ydt,
                            addr_space="Shared")
    y_allB = nc.dram_tensor("y_allB", [ncores * 128, rB * d], ydt,
                            addr_space="Shared")
    pool_in = nc.dram_tensor("pool_in", [g, d], f32)
    pool_out = nc.dram_tensor("pool_out", [g, d], f32, addr_space="Shared")

    y_rows = {"A": y_allA[:].rearrange("q (r dd) -> (q r) dd", dd=d),
              "B": y_allB[:].rearrange("q (r dd) -> (q r) dd", dd=d)}

    with tile.TileContext(nc) as tc:
        with (
            tc.tile_pool(name="const", bufs=1) as cp,
            tc.tile_pool(name="work", bufs=3) as wp,
            tc.tile_pool(name="gatp", bufs=3) as gp,
            tc.tile_pool(name="redp", bufs=2) as rp,
            tc.tile_pool(name="psA", bufs=3, space="PSUM") as psA,
            tc.tile_pool(name="psB", bufs=3, space="PSUM") as psB,
            tc.tile_pool(name="psP", bufs=1, space="PSUM") as psP,
        ):
            ident = cp.tile([128, 128], f32)
            make_identity(nc, ident[:])
            dinv_sb = cp.tile([128, r_], f32)
            nc.sync.dma_start(dinv_sb[:], dinv_in[:])
            dinv2_sb = cp.tile([128, r_], f32)
            nc.sync.dma_start(dinv2_sb[:], dinv2_in[:])
            mp_sb = cp.tile([128, r_ * g], f32)
            nc.sync.dma_start(mp_sb[:], mp_in[:])
            wf_sb = cp.tile([d, c_], f32)
            nc.sync.dma_start(wf_sb[:], wf_in[:])
            h_sb = cp.tile([128, r_ * d], f32, name="h_sb")
            y_sb = cp.tile([128, r_ * d], ydt)
            ystage = cp.tile([128, r_ * d], ydt, name="ystage")
            w0b_sb = cp.tile([d, d], bf16, name="w0b_sb")
            nc.sync.dma_start(w0b_sb[:], w0b_in[:])
            agg = [cp.tile([128, rh * d], ydt, name=f"agg{par}")
                   for par in range(2)]
            b_sbs = []
            if use_bias:
                for i in range(3):
                    b_sb = cp.tile([128, d], f32, name=f"b_sb{i}")
                    nc.sync.dma_start(b_sb[:], b_ins[i][:])
                    b_sbs.append(b_sb)
            if use_fbias:
                bf_sb = cp.tile([g, c_], f32)
                nc.sync.dma_start(bf_sb[:], bf_in[:])

            y3 = y_sb[:].rearrange("p (r dd) -> p r dd", dd=d)
            agg3 = [a[:].rearrange("p (r dd) -> p r dd", dd=d) for a in agg]
            gA = rA // 2

            def init_agg():
                # self-loop init, split per half so the H1 copy (and thus
                # H1 scatters) only depends on the first rA y chunks
                for par in range(2):
                    nc.vector.tensor_copy(agg3[par][:, :gA, :],
                                          y3[:, par:rA:2, :])
                    cntB = (r_ - rA + 1 - par) // 2
                    nc.vector.tensor_copy(agg3[par][:, gA:gA + cntB, :],
                                          y3[:, rA + par::2, :])

            blocks = {("A", 0): [], ("A", 1): [], ("B", 0): [], ("B", 1): []}
            for ci, ch in enumerate(chunks):
                blocks[(ch["tensor"], ch["half"])].append((ci, ch))

            def emit_chunks(l, blk):
                # chunked gather + DVE pre-reduce + SBUF CCE scatter-add
                for ci, ch in blk:
                    region, half = ch["region"], ch["half"]
                    csz = CSZ[region]
                    ncell = ch["ncell"]
                    nidx = ncell * csz
                    gt = wp.tile([128, nidx // 16], i16, tag="gidx",
                                 name=f"gi{l}_{ci}")
                    nc.sync.dma_start(
                        gt[:], gidx_in[:, ch["gpos"] // 16:
                                       (ch["gpos"] + nidx) // 16])
                    st = wp.tile([128, ncell // 16], i16, tag="sidx",
                                 name=f"si{l}_{ci}")
                    nc.sync.dma_start(
                        st[:], sidx_in[:, ch["spos"] // 16:
                                       (ch["spos"] + ncell) // 16])
                    gat = gp.tile([128, (nidx // 128) * d], ydt, tag="gat",
                                  name=f"gat{l}_{ci}")
                    nc.gpsimd.dma_gather(
                        out_ap=gat[:].rearrange("p (k dd) -> p k dd", dd=d),
                        in_ap=y_rows[ch["tensor"]][ch["lo"]:ch["hi"], :],
                        idxs_ap=gt[:], num_idxs=nidx, num_idxs_reg=nidx,
                        elem_size=d, single_packet=False)
                    surv = gat
                    k = nidx // 128
                    lvl = 0
                    while k > ncell // 128:
                        k //= 2
                        lvl += 1
                        nxt = rp.tile([128, k * d], ydt,
                                      tag=f"red_{region}L{lvl}",
                                      name=f"red{l}_{ci}_{k}")
                        s3 = surv[:].rearrange("p (k dd) -> p k dd", dd=d)
                        nc.vector.tensor_tensor(
                            out=nxt[:].rearrange("p (k dd) -> p k dd", dd=d),
                            in0=s3[:, 0::2, :], in1=s3[:, 1::2, :],
                            op=mybir.AluOpType.add)
                        surv = nxt
                    s3 = surv[:].rearrange("p (k dd) -> p k dd", dd=d)
                    if half == 0:
                        oap = [agg[0][:, :gA * d], agg[1][:, :gA * d]]
                    else:
                        oap = [agg[0][:, gA * d:], agg[1][:, gA * d:]]
                    off = 0
                    for sz in ch["pieces"]:
                        nc.gpsimd.dma_scatter_add(
                            out_ap=oap[0],
                            in_ap=s3[:, off // 128:(off + sz) // 128, :],
                            idxs_ap=st[:, off // 16:(off + sz) // 16],
                            num_idxs=sz, num_idxs_reg=sz,
                            elem_size=d,
                            sbuf_tokens_per_rank=128,
                            parity_reg=0,
                            out_ap_other=oap[1])
                        off += sz

            def p4_half(l, half):
                # h = relu(scale * agg); scale folds the next layer's dinv
                sc = dinv_sb if l == n_layers - 1 else dinv2_sb
                rlo, rhi = (0, rA) if half == 0 else (rA, r_)
                for r in range(rlo, rhi):
                    nc.scalar.activation(
                        out=h_sb[:, r * d:(r + 1) * d],
                        in_=agg3[r % 2][:, r // 2, :],
                        func=mybir.ActivationFunctionType.Relu,
                        scale=sc[:, r:r + 1])

            def p1_half(l, w_sb, half):
                # y(l) = h @ W for one half (h pre-scaled), then its
                # AllGather; groups of 4 chunks share one psum bank
                rlo, rhi = (0, rA) if half == 0 else (rA, r_)
                for r0 in range(rlo, rhi, 4):
                    nr = min(4, rhi - r0)
                    mm = psB.tile([128, 4 * d], f32, tag="mm",
                                  name=f"mm{l}_{r0}")
                    for t in range(nr):
                        r = r0 + t
                        tp = psA.tile([128, 128], f32, tag="tp",
                                      name=f"tp{l}_{r}")
                        nc.tensor.transpose(
                            out=tp[:], in_=h_sb[:, r * d:(r + 1) * d],
                            identity=ident[:])
                        hT = wp.tile([128, 128], f32, tag="hT",
                                     name=f"hT{l}_{r}")
                        nc.vector.tensor_copy(hT[:], tp[:])
                        nc.tensor.matmul(out=mm[:, t * d:(t + 1) * d],
                                         lhsT=hT[:], rhs=w_sb[:],
                                         start=True, stop=True)
                    nc.scalar.copy(
                        out=y_sb[:, r0 * d:(r0 + nr) * d],
                        in_=mm[:, :nr * d])
                if half == 0:
                    nc.gpsimd.dma_start(y_cA[:], y_sb[:, :rA * d])
                    nc.gpsimd.collective_compute(
                        "AllGather", mybir.AluOpType.bypass,
                        replica_groups=rg, ins=[y_cA[:]], outs=[y_allA[:]])
                else:
                    nc.gpsimd.dma_start(y_cB[:], y_sb[:, rA * d:])
                    nc.gpsimd.collective_compute(
                        "AllGather", mybir.AluOpType.bypass,
                        replica_groups=rg, ins=[y_cB[:]], outs=[y_allB[:]])

            for l in range(n_layers):
                # Layer 0: every core computes y1 = (dinv*x) @ W1 for ALL
                # owners from the replicated pre-scaled xT stream and writes
                # the slabs to the shared y_all tensors -- no AllGather.
                if l == 0:
                    h0 = (r_ + 1) // 2
                    for o in [0, 1, ncores] + list(range(2, ncores)):
                        # o < ncores: owner o's slab -> y_all (replicated
                        # stream); o == ncores: MY slab into y_sb for the
                        # self-loop init copy -- first, so the accumulator
                        # init does not gate the scatter pipeline.
                        my = o == ncores
                        stage = y_sb if (my or o % 2 == 0) else ystage
                        for half, (rlo, rcnt) in enumerate(
                                ((0, h0), (h0, r_ - h0))):
                            xs = rp.tile([128, h0 * d], bf16, tag="xs",
                                         name=f"xs{o}_{half}")
                            if my:
                                nc.sync.dma_start(
                                    xs[:, :rcnt * d],
                                    xT_in[:, rlo * 128:(rlo + rcnt) * 128])
                            else:
                                base = (o * r_ + rlo) * 128
                                nc.sync.dma_start(
                                    xs[:, :rcnt * d],
                                    xTf_in[:, base:base + rcnt * 128])
                            for r0 in range(0, rcnt, 4):
                                nr = min(4, rcnt - r0)
                                mm = psB.tile([128, 4 * d], f32, tag="mm",
                                              name=f"mm0_{o}_{half}_{r0}")
                                for t in range(nr):
                                    nc.tensor.matmul(
                                        out=mm[:, t * d:(t + 1) * d],
                                        lhsT=xs[:, (r0 + t) * d:
                                                (r0 + t + 1) * d],
                                        rhs=w0b_sb[:], start=True, stop=True)
                                dst = stage[:, (rlo + r0) * d:
                                            (rlo + r0 + nr) * d]
                                if (r0 // 4) % 2 == 0:
                                    nc.scalar.copy(out=dst, in_=mm[:, :nr * d])
                                else:
                                    nc.vector.tensor_copy(dst, mm[:, :nr * d])
                        if my:
                            # consume y_sb immediately: self-loop init of the
                            # accumulators, so the owner stream can reuse it
                            init_agg()
                        else:
                            nc.gpsimd.dma_start(
                                y_allA[o * 128:(o + 1) * 128, :],
                                stage[:, :rA * d])
                            nc.gpsimd.dma_start(
                                y_allB[o * 128:(o + 1) * 128, :],
                                stage[:, rA * d:])
                else:
                    # y(l) was computed and AllGathered at the tail of the
                    # previous layer; just (re)initialize the accumulators.
                    init_agg()
                # edge phase, ordered so the H1-dst cells finish first and
                # the A-src cells only need AllGather-A.  Layer 0 has no
                # collective dependencies at all, so it can finish ALL H1
                # cells at the halfway mark and launch AllGather-A(1) early.
                if l == 0:
                    # H1 cells complete at ~50%; the trailing (A,1) block
                    # must still precede p1_half(1,0), whose AllGather-A
                    # overwrites y_allA that those gathers read.
                    emit_chunks(l, blocks[("A", 0)] + blocks[("B", 0)]
                                + blocks[("A", 1)])
                else:
                    emit_chunks(l, blocks[("A", 0)] + blocks[("A", 1)]
                                + blocks[("B", 0)])
                p4_half(l, 0)
                if l < n_layers - 1:
                    w_sb = wp.tile([d, d], f32, tag="w", name=f"w_sb{l + 1}")
                    nc.sync.dma_start(w_sb[:], w_ins[l + 1][:])
                    # next layer's y for H1 + its AllGather overlap the H2
                    # edge work below and the H2 collective
                    p1_half(l + 1, w_sb, 0)
                emit_chunks(l, blocks[("B", 1)])
                p4_half(l, 1)
                if l < n_layers - 1:
                    p1_half(l + 1, w_sb, 1)

            # mean-pool via membership matmul, accumulated in one psum bank
            pp = psP.tile([g, d], f32)
            for r in range(r_):
                nc.tensor.matmul(out=pp[:], lhsT=mp_sb[:, r * g:(r + 1) * g],
                                 rhs=h_sb[:, r * d:(r + 1) * d],
                                 start=(r == 0), stop=(r == r_ - 1))
            pooled = wp.tile([g, d], f32, tag="pooled")
            nc.vector.tensor_copy(pooled[:], pp[:])
            nc.gpsimd.dma_start(pool_in[:], pooled[:])
            nc.gpsimd.collective_compute(
                "AllReduce", mybir.AluOpType.add, replica_groups=rg,
                ins=[pool_in[:]], outs=[pool_out[:]])
            pall = wp.tile([g, d], f32, tag="pall")
            nc.sync.dma_start(pall[:], pool_out[:])

            # FFN: logits = pooled @ Wf (+bf), then softmax over classes
            ptp = psA.tile([128, 128], f32, tag="tp", name="ptp")
            nc.tensor.transpose(out=ptp[:, :g], in_=pall[:],
                                identity=ident[:g, :g])
            pT = wp.tile([128, g], f32, tag="pT")
            nc.vector.tensor_copy(pT[:], ptp[:, :g])
            lg_ps = psB.tile([g, 4 * d], f32, tag="mm", name="lg_ps")
            nc.tensor.matmul(out=lg_ps[:, :c_], lhsT=pT[:], rhs=wf_sb[:],
                             start=True, stop=True)
            lg = wp.tile([g, c_], f32, tag="lg")
            if use_fbias:
                nc.vector.tensor_tensor(out=lg[:], in0=lg_ps[:, :c_],
                                        in1=bf_sb[:], op=mybir.AluOpType.add)
            else:
                nc.vector.tensor_copy(lg[:], lg_ps[:, :c_])
            mx = wp.tile([g, 1], f32, tag="mx")
            nc.vector.tensor_reduce(out=mx[:], in_=lg[:],
                                    axis=mybir.AxisListType.X,
                                    op=mybir.AluOpType.max)
            mxn = wp.tile([g, 1], f32, tag="mxn")
            nc.vector.tensor_scalar_mul(mxn[:], mx[:], -1.0)
            ex = wp.tile([g, c_], f32, tag="ex")
            nc.scalar.activation(out=ex[:], in_=lg[:],
                                 func=mybir.ActivationFunctionType.Exp,
                                 bias=mxn[:, :1])
            sm = wp.tile([g, 1], f32, tag="sm")
            nc.vector.tensor_reduce(out=sm[:], in_=ex[:],
                                    axis=mybir.AxisListType.X,
                                    op=mybir.AluOpType.add)
            rs = wp.tile([g, 1], f32, tag="rs")
            nc.vector.reciprocal(rs[:], sm[:])
            ot = wp.tile([g, c_], f32, tag="ot")
            nc.vector.tensor_scalar_mul(ot[:], ex[:], rs[:, :1])
            nc.gpsimd.dma_start(out_ext[:], ot[:])
    return nc


def run_gcn(x, edge_index, batch, ws, bs, wf, bf, *, n, e, g, d, c_,
            ncores=NCORES, trace=False, run=True, n_layers=3):
    prep = host_prep(x, edge_index, batch, n, g, d, ncores)
    use_bias = any(np.any(np.asarray(b) != 0) for b in bs)
    use_fbias = bool(np.any(np.asarray(bf) != 0))
    assert not use_bias

    nc = bacc.Bacc("TRN2", target_bir_lowering=False, debug=False,
                   num_devices=ncores)
    build_gcn(nc, r_=prep["r_"], rh=prep["rh"], chunks=prep["chunks"],
              total_gidx=prep["total_gidx"], total_sidx=prep["total_sidx"],
              rA=prep["rA"], rB=prep["rB"], d=d, g=g, c_=c_,
              ncores=ncores, use_bias=use_bias, use_fbias=use_fbias,
              n_layers=n_layers)
    nc.compile()

    in_maps = []
    for c in range(ncores):
        m = {
            "xTf_pr": prep["xTf_pr"],
            "xT_pr": prep["xT_pr"][c],
            "w0b": np.asarray(ws[0]).astype(
                mybir.dt.np(mybir.dt.bfloat16)),
            "dinv_pr": prep["dinv_pr"][c],
            "dinv2_pr": prep["dinv2_pr"][c],
            "gidx_pr": prep["gidx_pr"][c],
            "sidx_pr": prep["sidx_pr"][c],
            "mp_pr": prep["mp_pr"][c],
            "wf": np.asarray(wf, np.float32),
        }
        for i in range(3):
            m[f"w{i}"] = np.asarray(ws[i], np.float32)
        if use_fbias:
            m["bfr"] = np.broadcast_to(
                np.asarray(bf, np.float32), (g, c_)).copy()
        in_maps.append(m)

    if not run:
        return None, (None, nc, in_maps)
    res = run_bass_kernel_spmd(nc, in_maps, core_ids=list(range(ncores)),
                               trace=trace)
    return res.results[0]["out"].astype(np.float32), (res, nc, in_maps)


def bench_pjrt(nc, in_maps, ncores, iters=5):
    """Mirror bass2jax.run_bass_via_pjrt's multi-core path, but keep inputs
    device-resident and loop execution to time steady-state runs."""
    import time as _time
    import jax
    from jax.experimental.shard_map import shard_map
    from jax.sharding import Mesh, PartitionSpec
    from concourse import bass2jax as b2j
    import concourse.mybir as mb

    b2j.install_neuronx_cc_hook()
    partition_name = (nc.partition_id_tensor.name
                      if nc.partition_id_tensor else None)
    in_names, out_names, out_avals, zero_outs = [], [], [], []
    for alloc in nc.m.functions[0].allocations:
        if not isinstance(alloc, mb.MemoryLocationSet):
            continue
        name = alloc.memorylocations[0].name
        if alloc.kind == "ExternalInput":
            if name != partition_name:
                in_names.append(name)
        elif alloc.kind == "ExternalOutput":
            shape = tuple(alloc.tensor_shape)
            dtype = mb.dt.np(alloc.dtype)
            out_names.append(name)
            out_avals.append(jax.core.ShapedArray(shape, dtype))
            zero_outs.append(np.zeros(shape, dtype))
    n_params = len(in_names)
    n_outs = len(out_avals)
    in_names.extend(out_names)
    donate = tuple(range(n_params, n_params + n_outs))

    def _body(*args):
        outs = b2j._bass_exec_p.bind(
            *list(args), out_avals=tuple(out_avals), in_names=tuple(in_names),
            out_names=tuple(out_names), lowering_input_output_aliases=(),
            sim_require_finite=True, sim_require_nnan=True, nc=nc)
        return tuple(outs)

    devices = jax.devices()[:ncores]
    mesh = Mesh(np.asarray(devices), ("core",))
    sharded = jax.jit(
        shard_map(_body, mesh=mesh,
                  in_specs=(PartitionSpec("core"),) * (n_params + n_outs),
                  out_specs=(PartitionSpec("core"),) * n_outs,
                  check_rep=False),
        donate_argnums=donate, keep_unused=True)
    concat_in = [np.concatenate([np.asarray(in_maps[c][nm])
                                 for c in range(ncores)], axis=0)
                 for nm in in_names[:n_params]]
    sh_in = jax.sharding.NamedSharding(mesh, PartitionSpec("core"))
    dev_in = [jax.device_put(a, sh_in) for a in concat_in]

    times = []
    out_arrs = None
    for it in range(iters):
        zeros = [jax.device_put(
            np.zeros((ncores * z.shape[0], *z.shape[1:]), z.dtype), sh_in)
            for z in zero_outs]
        for z in zeros:
            z.block_until_ready()
        t0 = _time.perf_counter()
        out_arrs = sharded(*dev_in, *zeros)
        for o in out_arrs:
            o.block_until_ready()
        times.append(_time.perf_counter() - t0)
    res0 = {name: np.asarray(out_arrs[i]).reshape(
        ncores, *out_avals[i].shape)[0] for i, name in enumerate(out_names)}
    return res0, times


def kernel(x, edge_index, batch, W1, b1, W2, b2, W3, b3, Wf, bf):
    out, _ = run_gcn(np.asarray(x), np.asarray(edge_index), np.asarray(batch),
                     [W1, W2, W3], [b1, b2, b3], Wf, bf,
                     n=N_FULL, e=E_FULL, g=G_FULL, d=D_FULL, c_=C_FULL)
    return out


# revision 67
# speedup vs baseline: 5.3705x; 5.3705x over previous
"""3-layer GCN + mean-pool + FFN + softmax on 8 Trainium2 NeuronCores.

Strategy (node sharding, per the edge-partitioning hint):
  - Nodes are sharded across 8 cores by id range (12500 each); within a core,
    nodes occupy 12544 slots laid out as 128 partitions x 98 chunks.
  - Per layer: y = dinv * (h @ W) chunk-wise (PE transpose + matmul; the dinv
    pre-scale for the NEXT layer is folded into the relu epilogue so P1 is a
    plain matmul), AllGather of y (bf16) across cores, then edge aggregation:
    edges are grouped per (dst core, src window) and each destination's edge
    count is decomposed into quads/pairs/singles.  Quad cells gather 4
    same-dst messages into 4 adjacent columns of one partition, two strided
    DVE adds reduce them 4->1, and only the survivors go through the CCE
    scatter-add -- directly into SBUF accumulators (parity-split mode), not
    DRAM.  This cuts scatter descriptors ~2.3x and removes all DRAM RMW.
  - Graph mean-pool via a matmul with a host-built membership/count matrix,
    AllReduce of the [16,128] partial, then FFN + softmax on-chip.
"""
import numpy as np

import concourse.bass as bass
import concourse.mybir as mybir
import concourse.tile as tile
from concourse import bacc
from concourse.bass_utils import run_bass_kernel_spmd
from concourse.masks import make_identity

NCORES = 8
N_FULL, E_FULL, G_FULL, D_FULL, C_FULL = 100000, 1600000, 16, 128, 16

f32 = mybir.dt.float32
i32 = mybir.dt.int32
i16 = mybir.dt.int16

WIN = 25088          # = 2 owner slabs, so window gathers chase slab writes
CSZ = {"q": 4, "p": 2, "s": 1}
# max gather idx per chunk (gat tile = IDX_PER_CHUNK rows x 256B)
IDX_PER_CHUNK = 3072


def wrap16(a):
    w16 = a.reshape(-1, 16).T.copy()
    return np.tile(w16, (8, 1))


def host_prep(x, edge_index, batch, n, g, d, ncores):
    """Build per-core slot layouts, quad/pair/single gather cells, pooling.

    Slots are split into two halves (A: chunks < rA, B: the rest), with pad
    slots in BOTH halves, and y lives in two shared tensors so each half's
    AllGather can overlap the other half's edge work."""
    np_ = (n + ncores - 1) // ncores          # nodes per core
    rA = min(48, max(2, (np_ // 128 + 1) // 2 // 2 * 2))
    slotsA = rA * 128
    padA = max(1, min(44, slotsA // 16))
    realA = min(np_, slotsA - padA)
    realB = np_ - realA
    assert realB > 0, "half split needs nodes in both halves"
    rB = (realB + 16 + 127) // 128
    r_ = rA + rB
    slots = r_ * 128
    rh = (r_ + 1) // 2

    dst_full = np.concatenate([np.asarray(edge_index[1]),
                               np.arange(n, dtype=np.int64)])
    deg = np.bincount(dst_full, minlength=n).astype(np.float32)
    dinv = np.where(deg > 0, 1.0 / np.sqrt(np.maximum(deg, 1e-30)),
                    0.0).astype(np.float32)

    node_core = np.minimum(np.arange(n) // np_, ncores - 1)
    local = np.arange(n) - node_core * np_
    rank = np.where(local < realA, local, local - realA + slotsA)
    # y row of node i: half A rows are q*rA + r, half B rows q*rB + (r - rA)
    rr = rank // 128
    qq = node_core * 128 + rank % 128
    in_a = rr < rA
    yrow = np.where(in_a, qq * rA + rr, qq * rB + (rr - rA))

    # per-core slot-ordered x, xT, dinv, batch (pads zero / -1)
    x_slot = np.zeros((ncores, slots, d), np.float32)
    dinv_slot = np.zeros((ncores, slots), np.float32)
    batch_slot = np.full((ncores, slots), -1, np.int64)
    flat = node_core * slots + rank
    x_slot.reshape(ncores * slots, d)[flat] = np.asarray(x, np.float32)
    dinv_slot.reshape(-1)[flat] = dinv
    batch_slot.reshape(-1)[flat] = np.asarray(batch)

    def to_pr(a):  # [ncores, slots, ...] -> [ncores, 128, r_ * ...]
        rest = a.shape[2:]
        m = int(np.prod(rest)) if rest else 1
        return (a.reshape(ncores, r_, 128, m).transpose(0, 2, 1, 3)
                 .reshape(ncores, 128, r_ * m).copy())

    dinv_pr = to_pr(dinv_slot[..., None])
    dinv2_pr = dinv_pr * dinv_pr
    # full transposed PRE-SCALED x (dinv*x), owner-major, bf16: every core
    # computes y1 for ALL nodes locally (kills the layer-1 AllGather)
    import concourse.mybir as _mb
    bfnp = _mb.dt.np(_mb.dt.bfloat16)
    xs_slot = x_slot * dinv_slot[..., None]
    xTf = (xs_slot.reshape(ncores, r_, 128, d).transpose(3, 0, 1, 2)
           .reshape(d, ncores * r_ * 128).astype(bfnp))
    # per-core copy of its own slab (the "9th pass" recomputes y_sb locally,
    # since SPMD code cannot address "my" slab inside the replicated stream)
    xT_pr = (xs_slot.reshape(ncores, r_, 128, d).transpose(0, 3, 1, 2)
             .reshape(ncores, d, r_ * 128).astype(bfnp))

    cnt = np.bincount(np.asarray(batch), minlength=g).astype(np.float32)
    cntc = np.clip(cnt, 1.0, None)
    onehot = (batch_slot[..., None] == np.arange(g)[None, None, :]
              ).astype(np.float32)
    mp = onehot / cntc[None, None, :]
    mp_pr = to_pr(mp)

    # --- edge cells: per (dst half, src tensor-window), quad/pair/single ---
    # (self-loops are handled by the on-chip init copy of the core's own y)
    e_src = np.asarray(edge_index[0]).astype(np.int64)
    e_dst = np.asarray(edge_index[1]).astype(np.int64)
    dst_core = node_core[e_dst]
    dst_rank = rank[e_dst]
    src_row = yrow[e_src]
    src_in_a = in_a[e_src]

    # tensor-windows: split each tensor's row space into <=32767-row windows
    wins = []                                 # (tensor, lo, hi)
    for tname, rows in (("A", ncores * 128 * rA), ("B", ncores * 128 * rB)):
        nw = (rows + 32767) // 32768
        wsz = (rows + nw - 1) // nw
        wsz = (wsz + 127) // 128 * 128
        for k in range(nw):
            lo, hi = k * wsz, min((k + 1) * wsz, rows)
            if lo < hi:
                wins.append((tname, lo, hi))
    # edge -> window index
    src_wi = np.full(len(e_src), -1, np.int64)
    for wi, (tname, lo, hi) in enumerate(wins):
        m = (src_in_a if tname == "A" else ~src_in_a) \
            & (src_row >= lo) & (src_row < hi)
        src_wi[m] = wi
    assert (src_wi >= 0).all()
    dst_half = (dst_rank >= slotsA).astype(np.int64)
    # scatter pad ranks, half-relative
    pad_rel = {0: slotsA - 1, 1: slots - 1 - slotsA}

    # chunk emit order: A-src blocks first (they only need AllGather-A),
    # and within each src tensor the H1-dst cells first (so the H1 relu +
    # next AllGather-A can launch while H2 edge work continues).
    keys = []
    for tgrp in ("A", "B"):
        for half in (0, 1):
            for wi, (tname, lo, hi) in enumerate(wins):
                if tname == tgrp:
                    keys.append((half, wi))

    # per (core, key, region): cells (round j, half-relative rank, srcs)
    per_cwr = {}
    for c in range(ncores):
        m_c = dst_core == c
        sr_c = src_row[m_c]
        rk_c = dst_rank[m_c]
        wi_c = src_wi[m_c]
        hf_c = dst_half[m_c]
        for half, wi in keys:
            lo = wins[wi][1]
            mm = (wi_c == wi) & (hf_c == half)
            sr = sr_c[mm] - lo
            rk = rk_c[mm] - half * slotsA
            o = np.argsort(rk, kind="stable")
            sr, rk = sr[o], rk[o]
            uniq, starts, cnts = np.unique(rk, return_index=True,
                                           return_counts=True)
            for region in ("q", "p", "s"):
                per_cwr[(c, half, wi, region)] = []
            for u, s0, cn in zip(uniq, starts, cnts):
                srcs = sr[s0:s0 + cn]
                pos = 0
                nq = cn // 4
                for j in range(nq):
                    per_cwr[(c, half, wi, "q")].append((j, u,
                                                        srcs[pos:pos + 4]))
                    pos += 4
                if (cn - pos) >= 2:
                    per_cwr[(c, half, wi, "p")].append((0, u,
                                                        srcs[pos:pos + 2]))
                    pos += 2
                if cn - pos:
                    per_cwr[(c, half, wi, "s")].append((0, u,
                                                        srcs[pos:pos + 1]))

    # Common chunk structure across cores. For each (key, region): rounds
    # with common (max-over-core) sizes padded to 128 cells, split into
    # pieces and grouped into chunks of <= IDX_PER_CHUNK gather indices.
    chunks = []
    gpos = spos = 0
    for half, wi in keys:
        for region in ("q", "p", "s"):
            csz = CSZ[region]
            cells_pc = IDX_PER_CHUNK // csz
            nrounds = 0
            by_core = []
            for c in range(ncores):
                cells = per_cwr[(c, half, wi, region)]
                nr = 1 + max((j for j, _, _ in cells), default=-1)
                nrounds = max(nrounds, nr)
                by_core.append(cells)
            if nrounds == 0:
                continue
            rsz = np.zeros(nrounds, np.int64)
            for c in range(ncores):
                cnt_r = np.bincount([j for j, _, _ in by_core[c]],
                                    minlength=nrounds)
                rsz = np.maximum(rsz, cnt_r)
            rsz = (rsz + 127) // 128 * 128
            pieces = []
            for j, sz in enumerate(rsz):
                sz = int(sz)
                while sz > cells_pc:
                    pieces.append((j, cells_pc))
                    sz -= cells_pc
                if sz:
                    pieces.append((j, sz))
            cur, cur_cells = [], 0
            groups = []
            for j, sz in pieces:
                if cur and cur_cells + sz > cells_pc:
                    groups.append(cur)
                    cur, cur_cells = [], 0
                cur.append((j, sz))
                cur_cells += sz
            if cur:
                groups.append(cur)
            for grp in groups:
                ncell = sum(sz for _, sz in grp)
                tname, lo, hi = wins[wi]
                chunks.append(dict(half=half, wi=wi, tensor=tname,
                                   lo=lo, hi=hi, region=region,
                                   pieces=[sz for _, sz in grp],
                                   rounds=[j for j, _ in grp],
                                   gpos=gpos, spos=spos, ncell=ncell))
                gpos += ncell * csz
                spos += ncell
    total_gidx, total_sidx = gpos, spos

    # fill per-core index arrays (scatter pads go to the half's pad slot;
    # gather pads read row 0 of the window -- the value is irrelevant since
    # pad slots are zeroed by dinv==0 in the relu epilogue)
    gidx = np.zeros((ncores, total_gidx), np.int16)
    sidx = np.zeros((ncores, total_sidx), np.int16)
    for c in range(ncores):
        cursor = {}
        for ch in chunks:
            half, wi, region = ch["half"], ch["wi"], ch["region"]
            csz = CSZ[region]
            cells = per_cwr[(c, half, wi, region)]
            base_g, base_s = ch["gpos"], ch["spos"]
            cell_off = 0
            for j, sz in zip(ch["rounds"], ch["pieces"]):
                key = (half, wi, region, j)
                st = cursor.get(key, 0)
                sub = [cl for cl in cells if cl[0] == j][st:st + sz]
                cursor[key] = st + len(sub)
                for k, (_, rk, srcs) in enumerate(sub):
                    cc = cell_off + k
                    p, q = cc % 128, cc // 128
                    sidx[c, base_s + cc] = rk
                    for t in range(csz):
                        gidx[c, base_g + (q * csz + t) * 128 + p] = srcs[t]
                for k in range(len(sub), sz):
                    cc = cell_off + k
                    sidx[c, base_s + cc] = pad_rel[half]
                cell_off += sz
        # all cells consumed?
        for half, wi in keys:
            for region in ("q", "p", "s"):
                cells = per_cwr[(c, half, wi, region)]
                nr = 1 + max((j for j, _, _ in cells), default=-1)
                used = sum(cursor.get((half, wi, region, j), 0)
                           for j in range(nr))
                assert used == len(cells), (c, half, wi, region, used,
                                            len(cells))

    gidx_pr = np.stack([wrap16(gidx[c]) for c in range(ncores)])
    sidx_pr = np.stack([wrap16(sidx[c]) for c in range(ncores)])

    return dict(xTf_pr=xTf, xT_pr=xT_pr, dinv_pr=dinv_pr, dinv2_pr=dinv2_pr,
                mp_pr=mp_pr,
                gidx_pr=gidx_pr, sidx_pr=sidx_pr, chunks=chunks,
                total_gidx=total_gidx, total_sidx=total_sidx,
                r_=r_, rh=rh, rA=rA, rB=rB)


def build_gcn(nc, *, r_, rh, chunks, total_gidx, total_sidx, rA, rB, d, g,
              c_, ncores, use_bias, use_fbias, n_layers=3, ydt=None):
    if ydt is None:
        ydt = mybir.dt.bfloat16
    rg = [list(range(ncores))]

    bf16 = mybir.dt.bfloat16
    xTf_in = nc.dram_tensor("xTf_pr", [d, ncores * r_ * 128], bf16,
                            kind="ExternalInput")
    xT_in = nc.dram_tensor("xT_pr", [d, r_ * 128], bf16,
                           kind="ExternalInput")
    w0b_in = nc.dram_tensor("w0b", [d, d], bf16, kind="ExternalInput")
    dinv_in = nc.dram_tensor("dinv_pr", [128, r_], f32, kind="ExternalInput")
    dinv2_in = nc.dram_tensor("dinv2_pr", [128, r_], f32,
                              kind="ExternalInput")
    gidx_in = nc.dram_tensor("gidx_pr", [128, total_gidx // 16], i16,
                             kind="ExternalInput")
    sidx_in = nc.dram_tensor("sidx_pr", [128, total_sidx // 16], i16,
                             kind="ExternalInput")
    mp_in = nc.dram_tensor("mp_pr", [128, r_ * g], f32, kind="ExternalInput")
    w_ins = [nc.dram_tensor(f"w{i}", [d, d], f32, kind="ExternalInput")
             for i in range(3)]
    wf_in = nc.dram_tensor("wf", [d, c_], f32, kind="ExternalInput")
    b_ins = [nc.dram_tensor(f"b{i}", [128, d], f32, kind="ExternalInput")
             for i in range(3)] if use_bias else None
    bf_in = (nc.dram_tensor("bfr", [g, c_], f32, kind="ExternalInput")
             if use_fbias else None)
    out_ext = nc.dram_tensor("out", [g, c_], f32, kind="ExternalOutput")

    y_cA = nc.dram_tensor("y_cA", [128, rA * d], ydt)
    y_cB = nc.dram_tensor("y_cB", [128, rB * d], ydt)
    y_allA = nc.dram_tensor("y_allA", [ncores * 128, rA * d], ydt,
                            addr_space="Shared")
    y_allB = nc.dram_tensor("y_allB", [ncores * 128, rB * d], ydt,
                            addr_space="Shared")
    pool_in = nc.dram_tensor("pool_in", [g, d], f32)
    pool_out = nc.dram_tensor("pool_out", [g, d], f32, addr_space="Shared")

    y_rows = {"A": y_allA[:].rearrange("q (r dd) -> (q r) dd", dd=d),
              "B": y_allB[:].rearrange("q (r dd) -> (q r) dd", dd=d)}

    with tile.TileContext(nc) as tc:
        with (
            tc.tile_pool(name="const", bufs=1) as cp,
            tc.tile_pool(name="work", bufs=3) as wp,
            tc.tile_pool(name="gatp", bufs=3) as gp,
            tc.tile_pool(name="redp", bufs=2) as rp,
            tc.tile_pool(name="psA", bufs=3, space="PSUM") as psA,
            tc.tile_pool(name="psB", bufs=3, space="PSUM") as psB,
            tc.tile_pool(name="psP", bufs=1, space="PSUM") as psP,
        ):
            ident = cp.tile([128, 128], f32)
            make_identity(nc, ident[:])
            dinv_sb = cp.tile([128, r_], f32)
            nc.sync.dma_start(dinv_sb[:], dinv_in[:])
            dinv2_sb = cp.tile([128, r_], f32)
            nc.sync.dma_start(dinv2_sb[:], dinv2_in[:])
            mp_sb = cp.tile([128, r_ * g], f32)
            nc.sync.dma_start(mp_sb[:], mp_in[:])
            wf_sb = cp.tile([d, c_], f32)
            nc.sync.dma_start(wf_sb[:], wf_in[:])
            h_sb = cp.tile([128, r_ * d], f32, name="h_sb")
            y_sb = cp.tile([128, r_ * d], ydt)
            ystage = cp.tile([128, r_ * d], ydt, name="ystage")
            w0b_sb = cp.tile([d, d], bf16, name="w0b_sb")
            nc.sync.dma_start(w0b_sb[:], w0b_in[:])
            agg = [cp.tile([128, rh * d], ydt, name=f"agg{par}")
                   for par in range(2)]
            b_sbs = []
            if use_bias:
                for i in range(3):
                    b_sb = cp.tile([128, d], f32, name=f"b_sb{i}")
                    nc.sync.dma_start(b_sb[:], b_ins[i][:])
                    b_sbs.append(b_sb)
            if use_fbias:
                bf_sb = cp.tile([g, c_], f32)
                nc.sync.dma_start(bf_sb[:], bf_in[:])

            y3 = y_sb[:].rearrange("p (r dd) -> p r dd", dd=d)
            agg3 = [a[:].rearrange("p (r dd) -> p r dd", dd=d) for a in agg]
            gA = rA // 2

            def init_agg_half(half):
                # self-loop init, split per half so the H1 copy (and thus
                # H1 scatters) only depends on the first rA y chunks
                for par in range(2):
                    if half == 0:
                        nc.vector.tensor_copy(agg3[par][:, :gA, :],
                                              y3[:, par:rA:2, :])
                    else:
                        cntB = (r_ - rA + 1 - par) // 2
                        nc.vector.tensor_copy(agg3[par][:, gA:gA + cntB, :],
                                              y3[:, rA + par::2, :])

            blocks = {("A", 0): [], ("A", 1): [], ("B", 0): [], ("B", 1): []}
            for ci, ch in enumerate(chunks):
                blocks[(ch["tensor"], ch["half"])].append((ci, ch))

            def emit_chunks(l, blk):
                # chunked gather + DVE pre-reduce + SBUF CCE scatter-add
                for ci, ch in blk:
                    region, half = ch["region"], ch["half"]
                    csz = CSZ[region]
                    ncell = ch["ncell"]
                    nidx = ncell * csz
                    gt = wp.tile([128, nidx // 16], i16, tag="gidx",
                                 name=f"gi{l}_{ci}")
                    nc.sync.dma_start(
                        gt[:], gidx_in[:, ch["gpos"] // 16:
                                       (ch["gpos"] + nidx) // 16])
                    st = wp.tile([128, ncell // 16], i16, tag="sidx",
                                 name=f"si{l}_{ci}")
                    nc.sync.dma_start(
                        st[:], sidx_in[:, ch["spos"] // 16:
                                       (ch["spos"] + ncell) // 16])
                    gat = gp.tile([128, (nidx // 128) * d], ydt, tag="gat",
                                  name=f"gat{l}_{ci}")
                    nc.gpsimd.dma_gather(
                        out_ap=gat[:].rearrange("p (k dd) -> p k dd", dd=d),
                        in_ap=y_rows[ch["tensor"]][ch["lo"]:ch["hi"], :],
                        idxs_ap=gt[:], num_idxs=nidx, num_idxs_reg=nidx,
                        elem_size=d, single_packet=False)
                    surv = gat
                    k = nidx // 128
                    lvl = 0
                    while k > ncell // 128:
                        k //= 2
                        lvl += 1
                        nxt = rp.tile([128, k * d], ydt,
                                      tag=f"red_{region}L{lvl}",
                                      name=f"red{l}_{ci}_{k}")
                        s3 = surv[:].rearrange("p (k dd) -> p k dd", dd=d)
                        nc.vector.tensor_tensor(
                            out=nxt[:].rearrange("p (k dd) -> p k dd", dd=d),
                            in0=s3[:, 0::2, :], in1=s3[:, 1::2, :],
                            op=mybir.AluOpType.add)
                        surv = nxt
                    s3 = surv[:].rearrange("p (k dd) -> p k dd", dd=d)
                    if half == 0:
                        oap = [agg[0][:, :gA * d], agg[1][:, :gA * d]]
                    else:
                        oap = [agg[0][:, gA * d:], agg[1][:, gA * d:]]
                    off = 0
                    for sz in ch["pieces"]:
                        nc.gpsimd.dma_scatter_add(
                            out_ap=oap[0],
                            in_ap=s3[:, off // 128:(off + sz) // 128, :],
                            idxs_ap=st[:, off // 16:(off + sz) // 16],
                            num_idxs=sz, num_idxs_reg=sz,
                            elem_size=d,
                            sbuf_tokens_per_rank=128,
                            parity_reg=0,
                            out_ap_other=oap[1])
                        off += sz

            def p4_half(l, half):
                # h = relu(scale * agg); scale folds the next layer's dinv.
                # Alternate chunks between the Scalar and Vector engines
                # (DVE does scale+relu as one two-op tensor_scalar).
                sc = dinv_sb if l == n_layers - 1 else dinv2_sb
                rlo, rhi = (0, rA) if half == 0 else (rA, r_)
                for r in range(rlo, rhi):
                    if r % 2 == 0:
                        nc.scalar.activation(
                            out=h_sb[:, r * d:(r + 1) * d],
                            in_=agg3[r % 2][:, r // 2, :],
                            func=mybir.ActivationFunctionType.Relu,
                            scale=sc[:, r:r + 1])
                    else:
                        nc.vector.tensor_scalar(
                            out=h_sb[:, r * d:(r + 1) * d],
                            in0=agg3[r % 2][:, r // 2, :],
                            scalar1=sc[:, r:r + 1], scalar2=0.0,
                            op0=mybir.AluOpType.mult,
                            op1=mybir.AluOpType.max)

            def p1_half(l, w_sb, half):
                # y(l) = h @ W for one half (h pre-scaled), then its
                # AllGather; groups of 4 chunks share one psum bank
                rlo, rhi = (0, rA) if half == 0 else (rA, r_)
                for r0 in range(rlo, rhi, 4):
                    nr = min(4, rhi - r0)
                    mm = psB.tile([128, 4 * d], f32, tag="mm",
                                  name=f"mm{l}_{r0}")
                    for t in range(nr):
                        r = r0 + t
                        tp = psA.tile([128, 128], f32, tag="tp",
                                      name=f"tp{l}_{r}")
                        nc.tensor.transpose(
                            out=tp[:], in_=h_sb[:, r * d:(r + 1) * d],
                            identity=ident[:])
                        hT = wp.tile([128, 128], f32, tag="hT",
                                     name=f"hT{l}_{r}")
                        nc.vector.tensor_copy(hT[:], tp[:])
                        nc.tensor.matmul(out=mm[:, t * d:(t + 1) * d],
                                         lhsT=hT[:], rhs=w_sb[:],
                                         start=True, stop=True)
                    nc.scalar.copy(
                        out=y_sb[:, r0 * d:(r0 + nr) * d],
                        in_=mm[:, :nr * d])
                if half == 0:
                    nc.gpsimd.dma_start(y_cA[:], y_sb[:, :rA * d])
                    nc.gpsimd.collective_compute(
                        "AllGather", mybir.AluOpType.bypass,
                        replica_groups=rg, ins=[y_cA[:]], outs=[y_allA[:]])
                else:
                    nc.gpsimd.dma_start(y_cB[:], y_sb[:, rA * d:])
                    nc.gpsimd.collective_compute(
                        "AllGather", mybir.AluOpType.bypass,
                        replica_groups=rg, ins=[y_cB[:]], outs=[y_allB[:]])

            pp = psP.tile([g, d], f32)

            for l in range(n_layers):
                # Layer 0: every core computes y1 = (dinv*x) @ W1 for ALL
                # owners from the replicated pre-scaled xT stream and writes
                # the slabs to the shared y_all tensors -- no AllGather.
                if l == 0:
                    # Half-major stream: all owners' A-half slabs are written
                    # during the first pass, so A-window gathers can start at
                    # ~50% of the stream instead of its end.  The my-pass
                    # (o == ncores) leads each pass and feeds the self-loop
                    # init for that half.
                    for half, (rlo, rcnt) in ((0, (0, rA)),
                                              (1, (rA, r_ - rA))):
                        for o in [ncores] + list(range(ncores)):
                            my = o == ncores
                            stage = y_sb if (my or o % 2 == 0) else ystage
                            xs = rp.tile([128, (r_ - rA) * d], bf16,
                                         tag="xs", name=f"xs{o}_{half}")
                            if my:
                                nc.sync.dma_start(
                                    xs[:, :rcnt * d],
                                    xT_in[:, rlo * 128:(rlo + rcnt) * 128])
                            else:
                                base = (o * r_ + rlo) * 128
                                nc.sync.dma_start(
                                    xs[:, :rcnt * d],
                                    xTf_in[:, base:base + rcnt * 128])
                            for r0 in range(0, rcnt, 4):
                                nr = min(4, rcnt - r0)
                                mm = psB.tile([128, 4 * d], f32, tag="mm",
                                              name=f"mm0_{o}_{half}_{r0}")
                                for t in range(nr):
                                    nc.tensor.matmul(
                                        out=mm[:, t * d:(t + 1) * d],
                                        lhsT=xs[:, (r0 + t) * d:
                                                (r0 + t + 1) * d],
                                        rhs=w0b_sb[:], start=True, stop=True)
                                dst = stage[:, (rlo + r0) * d:
                                            (rlo + r0 + nr) * d]
                                if (r0 // 4) % 2 == 0:
                                    nc.scalar.copy(out=dst, in_=mm[:, :nr * d])
                                else:
                                    nc.vector.tensor_copy(dst, mm[:, :nr * d])
                            if my:
                                init_agg_half(half)
                            elif half == 0:
                                nc.gpsimd.dma_start(
                                    y_allA[o * 128:(o + 1) * 128, :],
                                    stage[:, :rA * d])
                            else:
                                nc.gpsimd.dma_start(
                                    y_allB[o * 128:(o + 1) * 128, :],
                                    stage[:, rA * d:])
                else:
                    # y(l) was computed and AllGathered at the tail of the
                    # previous layer; just (re)initialize the accumulators.
                    init_agg_half(0)
                    init_agg_half(1)
                # edge phase, ordered so the H1-dst cells finish first and
                # the A-src cells only need AllGather-A (which for layer 0
                # means only the stream's first pass)
                emit_chunks(l, blocks[("A", 0)] + blocks[("A", 1)]
                            + blocks[("B", 0)])
                p4_half(l, 0)
                if l < n_layers - 1:
                    w_sb = wp.tile([d, d], f32, tag="w", name=f"w_sb{l + 1}")
                    nc.sync.dma_start(w_sb[:], w_ins[l + 1][:])
                    # next layer's y for H1 + its AllGather overlap the H2
                    # edge work below and the H2 collective
                    p1_half(l + 1, w_sb, 0)
                else:
                    # mean-pool accumulation for the H1 chunks overlaps the
                    # last layer's H2 edge work
                    for r in range(rA):
                        nc.tensor.matmul(
                            out=pp[:], lhsT=mp_sb[:, r * g:(r + 1) * g],
                            rhs=h_sb[:, r * d:(r + 1) * d],
                            start=(r == 0), stop=False)
                emit_chunks(l, blocks[("B", 1)])
                p4_half(l, 1)
                if l < n_layers - 1:
                    p1_half(l + 1, w_sb, 1)
                else:
                    for r in range(rA, r_):
                        nc.tensor.matmul(
                            out=pp[:], lhsT=mp_sb[:, r * g:(r + 1) * g],
                            rhs=h_sb[:, r * d:(r + 1) * d],
                            start=False, stop=(r == r_ - 1))

            # mean-pool psum was accumulated inside the last layer's tail
            pooled = wp.tile([g, d], f32, tag="pooled")
            nc.vector.tensor_copy(pooled[:], pp[:])
            nc.gpsimd.dma_start(pool_in[:], pooled[:])
            nc.gpsimd.collective_compute(
                "AllReduce", mybir.AluOpType.add, replica_groups=rg,
                ins=[pool_in[:]], outs=[pool_out[:]])
            pall = wp.tile([g, d], f32, tag="pall")
            nc.sync.dma_start(pall[:], pool_out[:])

            # FFN: logits = pooled @ Wf (+bf), then softmax over classes
            ptp = psA.tile([128, 128], f32, tag="tp", name="ptp")
            nc.tensor.transpose(out=ptp[:, :g], in_=pall[:],
                                identity=ident[:g, :g])
            pT = wp.tile([128, g], f32, tag="pT")
            nc.vector.tensor_copy(pT[:], ptp[:, :g])
            lg_ps = psB.tile([g, 4 * d], f32, tag="mm", name="lg_ps")
            nc.tensor.matmul(out=lg_ps[:, :c_], lhsT=pT[:], rhs=wf_sb[:],
                             start=True, stop=True)
            lg = wp.tile([g, c_], f32, tag="lg")
            if use_fbias:
                nc.vector.tensor_tensor(out=lg[:], in0=lg_ps[:, :c_],
                                        in1=bf_sb[:], op=mybir.AluOpType.add)
            else:
                nc.vector.tensor_copy(lg[:], lg_ps[:, :c_])
            mx = wp.tile([g, 1], f32, tag="mx")
            nc.vector.tensor_reduce(out=mx[:], in_=lg[:],
                                    axis=mybir.AxisListType.X,
                                    op=mybir.AluOpType.max)
            mxn = wp.tile([g, 1], f32, tag="mxn")
            nc.vector.tensor_scalar_mul(mxn[:], mx[:], -1.0)
            ex = wp.tile([g, c_], f32, tag="ex")
            nc.scalar.activation(out=ex[:], in_=lg[:],
                                 func=mybir.ActivationFunctionType.Exp,
                                 bias=mxn[:, :1])
            sm = wp.tile([g, 1], f32, tag="sm")
            nc.vector.tensor_reduce(out=sm[:], in_=ex[:],
                                    axis=mybir.AxisListType.X,
                                    op=mybir.AluOpType.add)
            rs = wp.tile([g, 1], f32, tag="rs")
            nc.vector.reciprocal(rs[:], sm[:])
            ot = wp.tile([g, c_], f32, tag="ot")
            nc.vector.tensor_scalar_mul(ot[:], ex[:], rs[:, :1])
            nc.gpsimd.dma_start(out_ext[:], ot[:])
    return nc


def run_gcn(x, edge_index, batch, ws, bs, wf, bf, *, n, e, g, d, c_,
            ncores=NCORES, trace=False, run=True, n_layers=3):
    prep = host_prep(x, edge_index, batch, n, g, d, ncores)
    use_bias = any(np.any(np.asarray(b) != 0) for b in bs)
    use_fbias = bool(np.any(np.asarray(bf) != 0))
    assert not use_bias

    nc = bacc.Bacc("TRN2", target_bir_lowering=False, debug=False,
                   num_devices=ncores)
    build_gcn(nc, r_=prep["r_"], rh=prep["rh"], chunks=prep["chunks"],
              total_gidx=prep["total_gidx"], total_sidx=prep["total_sidx"],
              rA=prep["rA"], rB=prep["rB"], d=d, g=g, c_=c_,
              ncores=ncores, use_bias=use_bias, use_fbias=use_fbias,
              n_layers=n_layers)
    nc.compile()

    in_maps = []
    for c in range(ncores):
        m = {
            "xTf_pr": prep["xTf_pr"],
            "xT_pr": prep["xT_pr"][c],
            "w0b": np.asarray(ws[0]).astype(
                mybir.dt.np(mybir.dt.bfloat16)),
            "dinv_pr": prep["dinv_pr"][c],
            "dinv2_pr": prep["dinv2_pr"][c],
            "gidx_pr": prep["gidx_pr"][c],
            "sidx_pr": prep["sidx_pr"][c],
            "mp_pr": prep["mp_pr"][c],
            "wf": np.asarray(wf, np.float32),
        }
        for i in range(3):
            m[f"w{i}"] = np.asarray(ws[i], np.float32)
        if use_fbias:
            m["bfr"] = np.broadcast_to(
                np.asarray(bf, np.float32), (g, c_)).copy()
        in_maps.append(m)

    if not run:
        return None, (None, nc, in_maps)
    res = run_bass_kernel_spmd(nc, in_maps, core_ids=list(range(ncores)),
                               trace=trace)
    return res.results[0]["out"].astype(np.float32), (res, nc, in_maps)


def bench_pjrt(nc, in_maps, ncores, iters=5):
    """Mirror bass2jax.run_bass_via_pjrt's multi-core path, but keep inputs
    device-resident and loop execution to time steady-state runs."""
    import time as _time
    import jax
    from jax.experimental.shard_map import shard_map
    from jax.sharding import Mesh, PartitionSpec
    from concourse import bass2jax as b2j
    import concourse.mybir as mb

    b2j.install_neuronx_cc_hook()
    partition_name = (nc.partition_id_tensor.name
                      if nc.partition_id_tensor else None)
    in_names, out_names, out_avals, zero_outs = [], [], [], []
    for alloc in nc.m.functions[0].allocations:
        if not isinstance(alloc, mb.MemoryLocationSet):
            continue
        name = alloc.memorylocations[0].name
        if alloc.kind == "ExternalInput":
            if name != partition_name:
                in_names.append(name)
        elif alloc.kind == "ExternalOutput":
            shape = tuple(alloc.tensor_shape)
            dtype = mb.dt.np(alloc.dtype)
            out_names.append(name)
            out_avals.append(jax.core.ShapedArray(shape, dtype))
            zero_outs.append(np.zeros(shape, dtype))
    n_params = len(in_names)
    n_outs = len(out_avals)
    in_names.extend(out_names)
    donate = tuple(range(n_params, n_params + n_outs))

    def _body(*args):
        outs = b2j._bass_exec_p.bind(
            *list(args), out_avals=tuple(out_avals), in_names=tuple(in_names),
            out_names=tuple(out_names), lowering_input_output_aliases=(),
            sim_require_finite=True, sim_require_nnan=True, nc=nc)
        return tuple(outs)

    devices = jax.devices()[:ncores]
    mesh = Mesh(np.asarray(devices), ("core",))
    sharded = jax.jit(
        shard_map(_body, mesh=mesh,
                  in_specs=(PartitionSpec("core"),) * (n_params + n_outs),
                  out_specs=(PartitionSpec("core"),) * n_outs,
                  check_rep=False),
        donate_argnums=donate, keep_unused=True)
    concat_in = [np.concatenate([np.asarray(in_maps[c][nm])
                                 for c in range(ncores)], axis=0)
                 for nm in in_names[:n_params]]
    sh_in = jax.sharding.NamedSharding(mesh, PartitionSpec("core"))
    dev_in = [jax.device_put(a, sh_in) for a in concat_in]

    times = []
    out_arrs = None
    for it in range(iters):
        zeros = [jax.device_put(
            np.zeros((ncores * z.shape[0], *z.shape[1:]), z.dtype), sh_in)
            for z in zero_outs]
        for z in zeros:
            z.block_until_ready()
        t0 = _time.perf_counter()
        out_arrs = sharded(*dev_in, *zeros)
        for o in out_arrs:
            o.block_until_ready()
        times.append(_time.perf_counter() - t0)
    res0 = {name: np.asarray(out_arrs[i]).reshape(
        ncores, *out_avals[i].shape)[0] for i, name in enumerate(out_names)}
    return res0, times


def kernel(x, edge_index, batch, W1, b1, W2, b2, W3, b3, Wf, bf):
    out, _ = run_gcn(np.asarray(x), np.asarray(edge_index), np.asarray(batch),
                     [W1, W2, W3], [b1, b2, b3], Wf, bf,
                     n=N_FULL, e=E_FULL, g=G_FULL, d=D_FULL, c_=C_FULL)
    return out
